# revision 58
# baseline (speedup 1.0000x reference)
"""Local-strided block-sparse paged attention (decode) on 8 Trainium2 cores.

Sharding: the 64 (sequence, kv-head) pairs are bin-packed across 8 cores x
8 slots. For each pair, the 4 q-heads of the kv-head group share one
deduplicated K/V panel (union of the 4 heads' CSR rows), so each K/V block
is streamed from HBM once instead of up to 4 times. Panels are variable
length; slot k has the same chunk count on every core (max over the 8 pairs
assigned to that slot), keeping the program SPMD. The program is built per
slot-size signature and cached.

Host (numpy) resolves CSR + block_tables into bf16 gathered panels and
additive masks; the device does QK -> exp -> PV with per-head masks and
ships back the unnormalized PV output plus per-partition exp-sums; the
final normalization (a [128]-sum and divide per row) happens on host.
"""
import numpy as np

B, H, KVH, D, X = 16, 16, 4, 128, 4
HPG = H // KVH              # q-heads per kv-head group (4)
BLK, MAXB = 16, 256
NC_CORES = 8
NSLOT = 8                   # (seq, kv-group) pairs per core
SM_SCALE = 1.0 / float(np.sqrt(D))


_GROUP_CAP = 30
_BUFS = (4, 4, 6)   # (psum sc, psum ov, sbuf small)
_DMA_ALT = False    # alternate KV groups onto the gpsimd (SWDGE) queue


def _schedule(slot_nc):
    """Slot processing order (small head, then descending) and DMA groups.

    slot_nc is descending by construction. Process the smallest slot first so
    the first KV transfer is tiny (minimal head-wait before PE starts), then
    the rest in size order; group subsequent slots into <=_GROUP_CAP-chunk
    transfers for DMA efficiency.
    """
    proc = [NSLOT - 1] + list(range(NSLOT - 1))
    groups = [[proc[0]]]
    cur = 0
    for k in proc[1:-1]:
        if cur and cur + slot_nc[k] <= _GROUP_CAP:
            groups[-1].append(k)
            cur += slot_nc[k]
        else:
            groups.append([k])
            cur = slot_nc[k]
    groups.append([proc[-1]])   # small own-transfer tail -> short compute tail
    return proc, groups


def _build_device_program(slot_nc, reps=1, mode="full"):
    """slot_nc: tuple of per-slot chunk counts (shared across cores).

    mode: 'full' (real kernel) | 'dma' (transfers only) | 'compute'
    (tiny transfers, full instruction stream) — for bottleneck probing.
    """
    import concourse.bacc as bacc
    import concourse.mybir as mybir
    from concourse.tile import TileContext
    import contextlib

    f32 = mybir.dt.float32
    bf16 = mybir.dt.bfloat16
    tot = sum(slot_nc)
    RC = NSLOT * HPG            # result columns (32)
    # stream: q (RC cols) | per slot: K (128*nc) | V (128*nc) | mask
    # (8*nc bf16 = 4*nc f32 bitcast)
    W = RC + (2 * 128 + 8) * tot  # bf16 elems per partition line
    proc, groups = _schedule(slot_nc)

    nc = bacc.Bacc("TRN2", target_bir_lowering=False)
    kv = nc.dram_tensor("kv", [128, W], bf16, kind="ExternalInput")
    out = nc.dram_tensor("out", [128, 2 * RC], f32, kind="ExternalOutput")

    with TileContext(nc) as tc:
        with (
            tc.tile_pool(name="kv", bufs=1) as kvp,
            tc.tile_pool(name="small", bufs=_BUFS[2]) as sp,
            tc.tile_pool(name="ps_sc", bufs=_BUFS[0], space="PSUM") as pp_sc,
            tc.tile_pool(name="ps_ov", bufs=_BUFS[1], space="PSUM") as pp_ov,
            tc.tile_pool(name="io", bufs=1) as iop,
        ):
            rep_ctx = (
                tc.For_i(0, reps, 1, hint_engines=(
                    mybir.EngineType.PE, mybir.EngineType.SP,
                    mybir.EngineType.DVE, mybir.EngineType.Activation))
                if reps > 1 else contextlib.nullcontext()
            )
            with rep_ctx:
                osb = iop.tile([128, 2 * RC], f32, tag="osb")
                osb2 = osb[:, :RC]        # PV accumulators; exp-sums at RC+
                if mode == "dma":
                    nc.vector.memset(osb[:], 0.0)

                # dummy exp at t~0: the one-time ACT exp-table load (~2.7us)
                # overlaps the first KV transfer instead of the first real exp
                wa = sp.tile([128, 1], f32, tag="wa")
                nc.vector.memset(wa[:], 0.0)
                wb = sp.tile([128, 1], f32, tag="wb")
                nc.scalar.activation(
                    wb[:], wa[:], mybir.ActivationFunctionType.Exp)

                qt = None
                woff = 0
                for gi, grp in enumerate(groups):
                    gw = sum((2 * 128 + 8) * slot_nc[k] for k in grp)
                    if gi == 0:
                        gw += RC          # q vectors ride at the stream head
                    gt = kvp.tile([128, gw], bf16, tag=f"kv{grp[0]}")
                    deng = (nc.gpsimd if (_DMA_ALT and gi % 2 == 1)
                            else nc.sync)
                    if mode == "compute":
                        deng.dma_start(
                            out=gt[:, :128], in_=kv[:, woff:woff + 128])
                    elif gi == 0 or gi == len(groups) - 1:
                        # split head/tail groups at their slot's V boundary:
                        # QK only needs q|K|mask, so scores+exp overlap the
                        # V transfer (head: earlier start; tail: PV-only tail)
                        cut = (RC if gi == 0 else 0) + 136 * slot_nc[grp[0]]
                        deng.dma_start(
                            out=gt[:, :cut], in_=kv[:, woff:woff + cut])
                        deng.dma_start(
                            out=gt[:, cut:], in_=kv[:, woff + cut:woff + gw])
                    else:
                        deng.dma_start(out=gt[:], in_=kv[:, woff:woff + gw])
                    woff += gw
                    goff = 0
                    if gi == 0:
                        qt = gt            # q vectors are its first RC columns
                        goff = RC
                    for k in grp:
                        if mode == "dma":
                            continue
                        ncK = slot_nc[k]
                        slot_w = (2 * 128 + 8) * ncK
                        kvt = (gt[:, :slot_w] if mode == "compute"
                               else gt[:, goff:goff + slot_w])
                        goff += slot_w

                        # scores_T[t, c*4+hh] = sum_d K[d, c*128+t] * q_hh[d]
                        sc = pp_sc.tile([128, HPG * ncK], f32, tag="sc")
                        for c in range(ncK):
                            nc.tensor.matmul(
                                sc[:, HPG * c:HPG * (c + 1)],
                                kvt[:, 128 * c:128 * (c + 1)],
                                qt[:, HPG * k:HPG * (k + 1)],
                                start=True, stop=True,
                            )
                        # slot layout is K | mask | V: scores+exp can finish
                        # while the slot's V half is still streaming in
                        mask_ap = kvt[:, 128 * ncK:136 * ncK].bitcast(f32)
                        ssb = sp.tile([128, HPG * ncK], f32, tag="ssb")
                        nc.vector.tensor_add(ssb[:], sc[:], mask_ap)
                        p = sp.tile([128, HPG * ncK], bf16, tag="p")
                        nc.scalar.activation(
                            p[:], ssb[:], mybir.ActivationFunctionType.Exp,
                            scale=SM_SCALE,
                        )
                        # denom per head: view p as [128, hh, c], reduce over c
                        nc.vector.reduce_sum(
                            osb[:, RC + HPG * k:RC + HPG * (k + 1)],
                            p[:].rearrange("t (c h) -> t h c", h=HPG),
                            axis=mybir.AxisListType.X,
                        )

                        # out[d, hh] = sum_t P_T[t, hh] * V_T[t, d]
                        # (V stationary: V weight-loads are input-independent,
                        # so PE prefetches them while ACT computes p)
                        ov = pp_ov.tile([128, HPG], f32, tag="ov")
                        vbase = 136 * ncK
                        for c in range(ncK):
                            nc.tensor.matmul(
                                ov[:],
                                kvt[:, vbase + 128 * c:vbase + 128 * (c + 1)],
                                p[:, HPG * c:HPG * (c + 1)],
                                start=(c == 0), stop=(c == ncK - 1),
                            )
                        nc.vector.tensor_copy(
                            osb2[:, HPG * k:HPG * (k + 1)], ov[:])

                nc.sync.dma_start(out=out[:, :], in_=osb[:])
    nc.compile()
    return nc


_NC_CACHE = {}
_LAST_RES = None
_LAST_IN_MAPS = None
_LAST_SLOT_NC = None


def kernel(q, k_cache, v_cache, block_tables, context_lens, layout_crow, layout_col):
    import ml_dtypes
    from concourse.bass_utils import run_bass_kernel_spmd

    bf16 = ml_dtypes.bfloat16
    q = np.asarray(q, np.float32)
    k_cache = np.asarray(k_cache, np.float32)
    v_cache = np.asarray(v_cache, np.float32)
    block_tables = np.asarray(block_tables, np.int32)
    context_lens = np.asarray(context_lens, np.int32)
    layout_crow = np.asarray(layout_crow, np.int32)
    layout_col = np.asarray(layout_col, np.int32)

    q_pid = context_lens.astype(np.int64) - 1            # [B]
    pbid = q_pid // BLK

    # ---- plan: per (b,g) dedup union + sizes ----
    pairs = []                                           # (b, g, U, cols_per_head)
    sizes = np.empty(B * KVH, np.int64)
    for b in range(B):
        for g in range(KVH):
            cols_h = []
            for hh in range(HPG):
                h = HPG * g + hh
                s, e = layout_crow[h, pbid[b]], layout_crow[h, pbid[b] + 1]
                cols_h.append(layout_col[h, s:e])
            U = np.unique(np.concatenate(cols_h))
            if len(U) == 0:
                U = np.zeros(1, np.int32)
            nC = max(1, -(-(len(U) * BLK) // 128))
            sizes[len(pairs)] = nC
            pairs.append((b, g, U, cols_h))

    # bin-pack: rank pairs by size desc; slot k gets ranks [8k, 8k+8),
    # one per core; slot size = max of the group (= first of the group)
    order = np.argsort(-sizes, kind="stable")
    slot_nc = tuple(int(sizes[order[NC_CORES * k]]) for k in range(NSLOT))
    assign = [[int(order[NC_CORES * k + j]) for k in range(NSLOT)]
              for j in range(NC_CORES)]                  # [core][slot] -> pair idx

    kcb = k_cache.astype(bf16)
    vcb = v_cache.astype(bf16)
    tok16 = np.arange(BLK, dtype=np.int64)

    proc, _groups = _schedule(slot_nc)
    in_maps = []
    for core in range(NC_CORES):
        kv_parts = []
        q_cols = [None] * NSLOT
        for k in proc:
            b, g, U, cols_h = pairs[assign[core][k]]
            ncK = slot_nc[k]
            nB, nT = ncK * (128 // BLK), ncK * 128
            nU = len(U)
            U_pad = np.full(nB, U[0], np.int64)
            U_pad[:nU] = U
            btp = block_tables[b, U_pad].astype(np.int64)  # physical block ids

            kp = kcb[btp, g]                             # [nB,32,16,4]
            kp = kp.transpose(1, 3, 0, 2).reshape(128, nT)
            vp = vcb[btp, g]                             # [nB,128,16]
            vp = (vp.transpose(0, 2, 1).reshape(ncK, 128, 128)
                  .transpose(1, 0, 2).reshape(128, nT))

            real = np.arange(nB) < nU                    # [nB]
            pos_ok = (U_pad[:, None] * BLK + tok16[None, :]) <= q_pid[b]
            mask4 = np.empty((nB, BLK, HPG), bool)
            for hh in range(HPG):
                member = np.isin(U_pad, cols_h[hh]) & real
                mask4[:, :, hh] = member[:, None] & pos_ok
            madd = np.where(mask4, np.float32(0.0), np.float32(-1e9))
            # [nB,16,4] -> [nT,4] -> [nC,128,4] -> [128, nC*4] f32,
            # shipped as raw bytes inside the bf16 kv stream (device bitcasts)
            mp = np.ascontiguousarray(
                madd.reshape(nT, HPG).reshape(ncK, 128, HPG)
                .transpose(1, 0, 2).reshape(128, ncK * HPG))
            kv_parts += [kp, mp.view(bf16), vp]          # K | mask | V
            q_cols[k] = q[b, HPG * g:HPG * (g + 1)].T      # [128, 4]

        q_all = np.concatenate(q_cols, axis=1).astype(bf16)   # [128, RC]
        in_maps.append({
            "kv": np.ascontiguousarray(
                np.concatenate([q_all] + kv_parts, axis=1)),
        })

    global _LAST_RES, _LAST_IN_MAPS, _LAST_SLOT_NC
    if slot_nc not in _NC_CACHE:
        _NC_CACHE[slot_nc] = _build_device_program(slot_nc)
    nc = _NC_CACHE[slot_nc]
    _LAST_IN_MAPS = in_maps
    _LAST_SLOT_NC = slot_nc

    res = run_bass_kernel_spmd(nc, in_maps, core_ids=list(range(NC_CORES)))
    _LAST_RES = res
    RC = NSLOT * HPG
    out = np.empty((B, H, D), np.float32)
    for core in range(NC_CORES):
        o = res.results[core]["out"]                     # [128, 2*RC] fp32
        denom = o[:, RC:].sum(axis=0)                    # [RC]
        for k in range(NSLOT):
            b, g, _, _ = pairs[assign[core][k]]
            cols = slice(HPG * k, HPG * (k + 1))
            out[b, HPG * g:HPG * (g + 1)] = (
                o[:, cols] / denom[cols][None, :]).T
    return out
